# revision 3
# baseline (speedup 1.0000x reference)
"""MultiHeadAttention forward on 8 Trainium2 NeuronCores.

Sharding: batch x head-group. Core c handles batch b = c//4 and heads
4g..4g+3 where g = c%4 (tensor-parallel over the 16 heads, data-parallel
over batch 2). Each core:
  - projects Q,K (head-transposed layout [64d, S]) and V (natural [S, 64d])
    for its 4 heads from the full [S, E] inputs with f32r matmuls,
  - computes scores twice (natural [m,k] for the softmax/attn output and
    transposed [k,m] for the attn@V product; PE contracts over partitions
    so both layouts are needed),
  - softmax without max-subtraction (scores are O(1) here), normalization
    folded as a reciprocal-scale on the DVE,
  - writes its 4 heads of attn_weights (64 MiB) and a partial output
    projection [S, E] that the host sums across the 4 cores of its batch.

Biases: bq/bk are added on-device (per-partition adds in the transposed
layout). bv and bo enter the final output additively (attn rows sum to 1)
and are added on the host: out += bv @ Wo.T + bo.
"""
import numpy as np

B, S, E, H, D = 2, 2048, 1024, 16, 64
GH = 4            # heads per core
C = GH * D        # 256 local channels
NCORES = 8
MT = S // 128     # 16 m-tiles
KC = S // 512     # 4 k-chunks of 512
EC = E // 128     # 8 contraction chunks

_cache = {}


def _build():
    import concourse.bacc as bacc
    import concourse.mybir as mybir
    import concourse.tile as tile
    import concourse.masks as masks

    F32, F32R = mybir.dt.float32, mybir.dt.float32r
    AF, AX = mybir.ActivationFunctionType, mybir.AxisListType

    nc = bacc.Bacc("TRN2", target_bir_lowering=False, debug=False,
                   num_devices=NCORES)

    xq_d = nc.dram_tensor("xq", [E, S], F32R, kind="ExternalInput")
    xk_d = nc.dram_tensor("xk", [E, S], F32R, kind="ExternalInput")
    xv_d = nc.dram_tensor("xv", [E, S], F32R, kind="ExternalInput")
    wq_d = nc.dram_tensor("wq", [E, C], F32R, kind="ExternalInput")
    wk_d = nc.dram_tensor("wk", [E, C], F32R, kind="ExternalInput")
    wv_d = nc.dram_tensor("wv", [E, C], F32R, kind="ExternalInput")
    wo_d = nc.dram_tensor("wo", [GH, D, E], F32R, kind="ExternalInput")
    bq_d = nc.dram_tensor("bq2", [128, 2], F32, kind="ExternalInput")
    bk_d = nc.dram_tensor("bk2", [128, 2], F32, kind="ExternalInput")

    attn_d = nc.dram_tensor("attn", [GH, S, S], F32, kind="ExternalOutput")
    outp_d = nc.dram_tensor("outp", [S, E], F32, kind="ExternalOutput")

    with tile.TileContext(nc) as tc:
        with (
            tc.tile_pool(name="cst", bufs=1) as cst,
            tc.tile_pool(name="qkv", bufs=1) as qkv,
        ):
            ident = cst.tile([128, 128], F32, tag="ident")
            masks.make_identity(nc, ident[:])

            wo_sb = cst.tile([64, GH, E], F32R, tag="wo")
            for h in range(GH):
                nc.sync.dma_start(wo_sb[:, h, :], wo_d[h])
            bq_sb = cst.tile([128, 2], F32, tag="bq")
            bk_sb = cst.tile([128, 2], F32, tag="bk")
            nc.sync.dma_start(bq_sb[:], bq_d[:])
            nc.sync.dma_start(bk_sb[:], bk_d[:])

            QT = qkv.tile([128, 2, S], F32R, tag="QT")   # c=ct*128+p, m
            KT = qkv.tile([128, 2, S], F32R, tag="KT")
            Vn = qkv.tile([128, MT, C], F32R, tag="Vn")  # k=mt*128+p, c

            # ---- Phase 1: projections (x streamed in m-halves) ----
            with (
                tc.tile_pool(name="xt", bufs=1) as xt_pool,
                tc.tile_pool(name="wqk", bufs=1) as wqk,
                tc.tile_pool(name="pp", bufs=8, space="PSUM") as pp,
            ):
                for xd, wd, dst, bias in (
                    (xq_d, wq_d, QT, bq_sb), (xk_d, wk_d, KT, bk_sb),
                ):
                    w = wqk.tile([128, EC, C], F32R, tag="w", name="w")
                    for e in range(EC):
                        nc.sync.dma_start(w[:, e, :], wd[e * 128:(e + 1) * 128, :])
                    for half in range(2):
                        m0h = half * 1024
                        xt = xt_pool.tile([128, EC, 1024], F32R, tag="xt",
                                          name="xt")
                        for e in range(EC):
                            nc.sync.dma_start(
                                xt[:, e, :],
                                xd[e * 128:(e + 1) * 128, m0h:m0h + 1024])
                        ps = [pp.tile([128, 512], F32, tag="pp", name=f"pp{i}")
                              for i in range(4)]
                        for e in range(EC):
                            for ct in range(2):
                                for mcl in range(2):
                                    nc.tensor.matmul(
                                        ps[ct * 2 + mcl][:],
                                        w[:, e, ct * 128:(ct + 1) * 128],
                                        xt[:, e, mcl * 512:(mcl + 1) * 512],
                                        start=(e == 0), stop=(e == EC - 1),
                                    )
                        for ct in range(2):
                            for mcl in range(2):
                                m0 = m0h + mcl * 512
                                nc.vector.tensor_scalar_add(
                                    dst[:, ct, m0:m0 + 512],
                                    ps[ct * 2 + mcl][:], bias[:, ct:ct + 1],
                                )

                w = wqk.tile([128, EC, C], F32R, tag="w", name="w")
                for e in range(EC):
                    nc.sync.dma_start(w[:, e, :], wv_d[e * 128:(e + 1) * 128, :])
                for half in range(2):
                    m0h = half * 1024
                    xt = xt_pool.tile([128, EC, 1024], F32R, tag="xt",
                                      name="xt")
                    for e in range(EC):
                        nc.sync.dma_start(
                            xt[:, e, :],
                            xv_d[e * 128:(e + 1) * 128, m0h:m0h + 1024])
                    ps = [pp.tile([128, 512], F32, tag="pp", name=f"pp{i}")
                          for i in range(8)]
                    for e in range(EC):
                        for mt8 in range(8):
                            nc.tensor.matmul(
                                ps[mt8][:, 0:C],
                                xt[:, e, mt8 * 128:(mt8 + 1) * 128],
                                w[:, e, :],
                                start=(e == 0), stop=(e == EC - 1),
                            )
                    for mt8 in range(8):
                        nc.vector.tensor_copy(
                            Vn[:, half * 8 + mt8, :], ps[mt8][:, 0:C])

            # ---- Phase 2: attention per head ----
            with (
                tc.tile_pool(name="attn_p", bufs=3) as attn_p,
                tc.tile_pool(name="attnT_p", bufs=3) as attnT_p,
                tc.tile_pool(name="sums_p", bufs=2) as sums_p,
                tc.tile_pool(name="row_p", bufs=1) as row_p,
                tc.tile_pool(name="sc", bufs=4, space="PSUM") as sc,
                tc.tile_pool(name="ot", bufs=4, space="PSUM") as ot,
            ):
                outT = qkv.tile([64, GH, S], F32R, tag="outT")
                for h in range(GH):
                    ct_h, hb = h // 2, (h % 2) * 64

                    def qs(j, n):  # [64, n] slice of Q^T for this head
                        return QT[hb:hb + 64, ct_h, j:j + n]

                    def ks(j, n):
                        return KT[hb:hb + 64, ct_h, j:j + n]

                    # (a) scores -> exp -> normalize -> attn output
                    sums_h = sums_p.tile([128, MT], F32, tag="sums")
                    for mt in range(MT):
                        att = attn_p.tile([128, S], F32, tag="att")
                        s4 = sums_p.tile([128, KC], F32, tag="s4")
                        for kc in range(KC):
                            p = sc.tile([128, 512], F32, tag="sc")
                            nc.tensor.matmul(
                                p[:], qs(mt * 128, 128), ks(kc * 512, 512))
                            nc.scalar.activation(
                                att[:, kc * 512:(kc + 1) * 512], p[:],
                                AF.Exp, scale=0.125,
                                accum_out=s4[:, kc:kc + 1],
                            )
                        nc.vector.reduce_sum(
                            sums_h[:, mt:mt + 1], s4[:], axis=AX.X)
                        rcp = sums_p.tile([128, 1], F32, tag="rcp")
                        nc.vector.reciprocal(rcp[:], sums_h[:, mt:mt + 1])
                        nc.vector.tensor_scalar_mul(att[:], att[:], rcp[:])
                        nc.sync.dma_start(
                            attn_d[h, mt * 128:(mt + 1) * 128, :], att[:])

                    # (b) transposed reciprocal row, broadcast over partitions
                    ptr = sc.tile([16, 128], F32, tag="sc")
                    nc.tensor.transpose(ptr[:], sums_h[:], ident[:])
                    recipT = sums_p.tile([16, 128], F32, tag="recipT")
                    nc.vector.reciprocal(recipT[:], ptr[:])
                    row = row_p.tile([1, S], F32, tag="row")
                    nc.sync.dma_start(
                        row[:].rearrange("a (b c) -> a b c", b=MT),
                        recipT[:].rearrange("a (o b) -> a o b", o=1),
                    )
                    rbc = row_p.tile([64, S], F32, tag="rbc")
                    nc.gpsimd.partition_broadcast(rbc[:], row[0:1, :])

                    # (c) transposed scores -> exp -> attn @ V (unnormalized)
                    ots = [ot.tile([64, 512], F32, tag="ot", name=f"ot{i}") for i in range(KC)]
                    for kt in range(MT):
                        attT = attnT_p.tile([128, S], F32R, tag="attT")
                        for mc in range(KC):
                            p = sc.tile([128, 512], F32, tag="sc")
                            nc.tensor.matmul(
                                p[:], ks(kt * 128, 128), qs(mc * 512, 512))
                            nc.scalar.activation(
                                attT[:, mc * 512:(mc + 1) * 512], p[:],
                                AF.Exp, scale=0.125)
                        for mc in range(KC):
                            nc.tensor.matmul(
                                ots[mc][:],
                                Vn[:, kt, h * D:(h + 1) * D],
                                attT[:, mc * 512:(mc + 1) * 512],
                                start=(kt == 0), stop=(kt == MT - 1),
                            )
                    # (d) normalize columns by 1/rowsum, store head output^T
                    for mc in range(KC):
                        nc.vector.tensor_tensor(
                            outT[0:64, h, mc * 512:(mc + 1) * 512],
                            ots[mc][:], rbc[0:64, mc * 512:(mc + 1) * 512],
                            mybir.AluOpType.mult,
                        )

            # ---- Phase 3: output projection (partial; host sums cores) ----
            with (
                tc.tile_pool(name="osb", bufs=3) as osb,
                tc.tile_pool(name="po", bufs=4, space="PSUM") as po,
            ):
                for mt in range(MT):
                    oc = osb.tile([128, E], F32, tag="oc")
                    for nh in range(2):
                        p = po.tile([128, 512], F32, tag="po")
                        for h in range(GH):
                            nc.tensor.matmul(
                                p[:],
                                outT[0:64, h, mt * 128:(mt + 1) * 128],
                                wo_sb[:, h, nh * 512:(nh + 1) * 512],
                                start=(h == 0), stop=(h == GH - 1),
                            )
                        nc.vector.tensor_copy(
                            oc[:, nh * 512:(nh + 1) * 512], p[:])
                    nc.sync.dma_start(
                        outp_d[mt * 128:(mt + 1) * 128, :], oc[:])

    nc.compile()
    return nc


def _get_nc():
    if "nc" not in _cache:
        _cache["nc"] = _build()
    return _cache["nc"]


def _make_in_maps(query, key, value, Wq, bq, Wk, bk, Wv, bv, Wo, bo):
    f32 = np.float32
    query = np.asarray(query, f32)
    key = np.asarray(key, f32)
    value = np.asarray(value, f32)
    Wq, Wk, Wv, Wo = (np.asarray(a, f32) for a in (Wq, Wk, Wv, Wo))
    bq, bk = np.asarray(bq, f32), np.asarray(bk, f32)

    xT = {}
    for b in range(B):
        xT[b] = (
            np.ascontiguousarray(query[b].T),
            np.ascontiguousarray(key[b].T),
            np.ascontiguousarray(value[b].T),
        )
    in_maps = []
    for core in range(NCORES):
        b, g = core // (NCORES // B), core % (NCORES // B)
        cols = slice(C * g, C * (g + 1))
        xq, xk, xv = xT[b]
        in_maps.append({
            "xq": xq, "xk": xk, "xv": xv,
            "wq": np.ascontiguousarray(Wq[cols, :].T),
            "wk": np.ascontiguousarray(Wk[cols, :].T),
            "wv": np.ascontiguousarray(Wv[cols, :].T),
            "wo": np.ascontiguousarray(Wo[:, cols].T.reshape(GH, D, E)),
            "bq2": np.ascontiguousarray(bq[cols].reshape(2, 128).T),
            "bk2": np.ascontiguousarray(bk[cols].reshape(2, 128).T),
        })
    return in_maps


def kernel_run(trace=False, **inputs):
    """Returns ((output, attn_weights), exec_time_ns_or_None)."""
    from concourse.bass_utils import run_bass_kernel_spmd
    nc = _get_nc()
    in_maps = _make_in_maps(**inputs)
    r = run_bass_kernel_spmd(nc, in_maps, list(range(NCORES)), trace=trace)
    res = r.results
    f32 = np.float32
    out = np.zeros((B, S, E), f32)
    attn = np.empty((B, H, S, S), f32)
    for core in range(NCORES):
        b, g = core // (NCORES // B), core % (NCORES // B)
        out[b] += res[core]["outp"]
        attn[b, GH * g:GH * (g + 1)] = res[core]["attn"]
    bv = np.asarray(inputs["bv"], f32)
    bo = np.asarray(inputs["bo"], f32)
    Wo = np.asarray(inputs["Wo"], f32)
    out += (bv @ Wo.T + bo)[None, None, :]
    return (out, attn), r.exec_time_ns


def kernel(**inputs):
    return kernel_run(trace=False, **inputs)[0]


# revision 4
# speedup vs baseline: 1.1267x; 1.1267x over previous
"""MultiHeadAttention forward on 8 Trainium2 NeuronCores.

Sharding: batch x head-group. Core c handles batch b = c//4 and heads
4g..4g+3 where g = c%4 (tensor-parallel over the 16 heads, data-parallel
over batch 2). Each core:
  - projects Q,K (head-transposed layout [64d, S]) and V (natural [S, 64d])
    for its 4 heads from the full [S, E] inputs with f32r matmuls,
  - computes scores twice (natural [m,k] for the softmax/attn output and
    transposed [k,m] for the attn@V product; PE contracts over partitions
    so both layouts are needed),
  - softmax without max-subtraction (scores are O(1) here), normalization
    folded as a reciprocal-scale on the DVE,
  - writes its 4 heads of attn_weights (64 MiB) and a partial output
    projection [S, E] that the host sums across the 4 cores of its batch.

Biases: bq/bk are added on-device (per-partition adds in the transposed
layout). bv and bo enter the final output additively (attn rows sum to 1)
and are added on the host: out += bv @ Wo.T + bo.
"""
import numpy as np

B, S, E, H, D = 2, 2048, 1024, 16, 64
GH = 4            # heads per core
C = GH * D        # 256 local channels
NCORES = 8
MT = S // 128     # 16 m-tiles
KC = S // 512     # 4 k-chunks of 512
EC = E // 128     # 8 contraction chunks

_cache = {}


def _build():
    import concourse.bacc as bacc
    import concourse.mybir as mybir
    import concourse.tile as tile
    import concourse.masks as masks

    F32, F32R = mybir.dt.float32, mybir.dt.float32r
    BF16 = mybir.dt.bfloat16
    AF, AX = mybir.ActivationFunctionType, mybir.AxisListType

    nc = bacc.Bacc("TRN2", target_bir_lowering=False, debug=False,
                   num_devices=NCORES)

    xq_d = nc.dram_tensor("xq", [E, S], BF16, kind="ExternalInput")
    xk_d = nc.dram_tensor("xk", [E, S], BF16, kind="ExternalInput")
    xv_d = nc.dram_tensor("xv", [E, S], BF16, kind="ExternalInput")
    wq_d = nc.dram_tensor("wq", [E, C], BF16, kind="ExternalInput")
    wk_d = nc.dram_tensor("wk", [E, C], BF16, kind="ExternalInput")
    wv_d = nc.dram_tensor("wv", [E, C], BF16, kind="ExternalInput")
    wo_d = nc.dram_tensor("wo", [GH, D, E], BF16, kind="ExternalInput")
    bq_d = nc.dram_tensor("bq2", [128, 2], F32, kind="ExternalInput")
    bk_d = nc.dram_tensor("bk2", [128, 2], F32, kind="ExternalInput")

    attn_d = nc.dram_tensor("attn", [GH, S, S], F32, kind="ExternalOutput")
    outp_d = nc.dram_tensor("outp", [S, E], F32, kind="ExternalOutput")

    with tile.TileContext(nc) as tc:
        with (
            tc.tile_pool(name="cst", bufs=1) as cst,
            tc.tile_pool(name="qkv", bufs=1) as qkv,
        ):
            ident = cst.tile([128, 128], F32, tag="ident")
            masks.make_identity(nc, ident[:])

            wo_sb = cst.tile([64, GH, E], BF16, tag="wo")
            for h in range(GH):
                nc.sync.dma_start(wo_sb[:, h, :], wo_d[h])
            bq_sb = cst.tile([128, 2], F32, tag="bq")
            bk_sb = cst.tile([128, 2], F32, tag="bk")
            nc.sync.dma_start(bq_sb[:], bq_d[:])
            nc.sync.dma_start(bk_sb[:], bk_d[:])

            QT = qkv.tile([128, 2, S], BF16, tag="QT")   # c=ct*128+p, m
            KT = qkv.tile([128, 2, S], BF16, tag="KT")
            Vn = qkv.tile([128, MT, C], BF16, tag="Vn")  # k=mt*128+p, c

            # ---- Phase 1: projections (x streamed in m-halves) ----
            with (
                tc.tile_pool(name="xt", bufs=1) as xt_pool,
                tc.tile_pool(name="wqk", bufs=1) as wqk,
                tc.tile_pool(name="pp", bufs=8, space="PSUM") as pp,
            ):
                for xd, wd, dst, bias in (
                    (xq_d, wq_d, QT, bq_sb), (xk_d, wk_d, KT, bk_sb),
                ):
                    w = wqk.tile([128, EC, C], BF16, tag="w", name="w")
                    for e in range(EC):
                        nc.sync.dma_start(w[:, e, :], wd[e * 128:(e + 1) * 128, :])
                    for half in range(2):
                        m0h = half * 1024
                        xt = xt_pool.tile([128, EC, 1024], BF16, tag="xt",
                                          name="xt")
                        for e in range(EC):
                            nc.sync.dma_start(
                                xt[:, e, :],
                                xd[e * 128:(e + 1) * 128, m0h:m0h + 1024])
                        ps = [pp.tile([128, 512], F32, tag="pp", name=f"pp{i}")
                              for i in range(4)]
                        for e in range(EC):
                            for ct in range(2):
                                for mcl in range(2):
                                    nc.tensor.matmul(
                                        ps[ct * 2 + mcl][:],
                                        w[:, e, ct * 128:(ct + 1) * 128],
                                        xt[:, e, mcl * 512:(mcl + 1) * 512],
                                        start=(e == 0), stop=(e == EC - 1),
                                    )
                        for ct in range(2):
                            for mcl in range(2):
                                m0 = m0h + mcl * 512
                                nc.vector.tensor_scalar_add(
                                    dst[:, ct, m0:m0 + 512],
                                    ps[ct * 2 + mcl][:], bias[:, ct:ct + 1],
                                )

                w = wqk.tile([128, EC, C], BF16, tag="w", name="w")
                for e in range(EC):
                    nc.sync.dma_start(w[:, e, :], wv_d[e * 128:(e + 1) * 128, :])
                for half in range(2):
                    m0h = half * 1024
                    xt = xt_pool.tile([128, EC, 1024], BF16, tag="xt",
                                      name="xt")
                    for e in range(EC):
                        nc.sync.dma_start(
                            xt[:, e, :],
                            xv_d[e * 128:(e + 1) * 128, m0h:m0h + 1024])
                    ps = [pp.tile([128, 512], F32, tag="pp", name=f"pp{i}")
                          for i in range(8)]
                    for e in range(EC):
                        for mt8 in range(8):
                            nc.tensor.matmul(
                                ps[mt8][:, 0:C],
                                xt[:, e, mt8 * 128:(mt8 + 1) * 128],
                                w[:, e, :],
                                start=(e == 0), stop=(e == EC - 1),
                            )
                    for mt8 in range(8):
                        nc.vector.tensor_copy(
                            Vn[:, half * 8 + mt8, :], ps[mt8][:, 0:C])

            # ---- Phase 2: attention per head ----
            with (
                tc.tile_pool(name="attn_p", bufs=3) as attn_p,
                tc.tile_pool(name="attnT_p", bufs=3) as attnT_p,
                tc.tile_pool(name="sums_p", bufs=2) as sums_p,
                tc.tile_pool(name="row_p", bufs=1) as row_p,
                tc.tile_pool(name="sc", bufs=4, space="PSUM") as sc,
                tc.tile_pool(name="ot", bufs=4, space="PSUM") as ot,
            ):
                outT = qkv.tile([64, GH, S], BF16, tag="outT")
                for h in range(GH):
                    ct_h, hb = h // 2, (h % 2) * 64

                    def qs(j, n):  # [64, n] slice of Q^T for this head
                        return QT[hb:hb + 64, ct_h, j:j + n]

                    def ks(j, n):
                        return KT[hb:hb + 64, ct_h, j:j + n]

                    # (a) scores -> exp -> normalize -> attn output
                    sums_h = sums_p.tile([128, MT], F32, tag="sums")
                    for mt in range(MT):
                        att = attn_p.tile([128, S], F32, tag="att")
                        s4 = sums_p.tile([128, KC], F32, tag="s4")
                        for kc in range(KC):
                            p = sc.tile([128, 512], F32, tag="sc")
                            nc.tensor.matmul(
                                p[:], qs(mt * 128, 128), ks(kc * 512, 512))
                            nc.scalar.activation(
                                att[:, kc * 512:(kc + 1) * 512], p[:],
                                AF.Exp, scale=0.125,
                                accum_out=s4[:, kc:kc + 1],
                            )
                        nc.vector.reduce_sum(
                            sums_h[:, mt:mt + 1], s4[:], axis=AX.X)
                        rcp = sums_p.tile([128, 1], F32, tag="rcp")
                        nc.vector.reciprocal(rcp[:], sums_h[:, mt:mt + 1])
                        nc.vector.tensor_scalar_mul(att[:], att[:], rcp[:])
                        nc.sync.dma_start(
                            attn_d[h, mt * 128:(mt + 1) * 128, :], att[:])

                    # (b) transposed reciprocal row, broadcast over partitions
                    ptr = sc.tile([16, 128], F32, tag="sc")
                    nc.tensor.transpose(ptr[:], sums_h[:], ident[:])
                    recipT = sums_p.tile([16, 128], F32, tag="recipT")
                    nc.vector.reciprocal(recipT[:], ptr[:])
                    row = row_p.tile([1, S], F32, tag="row")
                    nc.sync.dma_start(
                        row[:].rearrange("a (b c) -> a b c", b=MT),
                        recipT[:].rearrange("a (o b) -> a o b", o=1),
                    )
                    rbc = row_p.tile([64, S], F32, tag="rbc")
                    nc.gpsimd.partition_broadcast(rbc[:], row[0:1, :])

                    # (c) transposed scores -> exp -> attn @ V (unnormalized)
                    ots = [ot.tile([64, 512], F32, tag="ot", name=f"ot{i}") for i in range(KC)]
                    for kt in range(MT):
                        attT = attnT_p.tile([128, S], BF16, tag="attT")
                        for mc in range(KC):
                            p = sc.tile([128, 512], F32, tag="sc")
                            nc.tensor.matmul(
                                p[:], ks(kt * 128, 128), qs(mc * 512, 512))
                            nc.scalar.activation(
                                attT[:, mc * 512:(mc + 1) * 512], p[:],
                                AF.Exp, scale=0.125)
                        for mc in range(KC):
                            nc.tensor.matmul(
                                ots[mc][:],
                                Vn[:, kt, h * D:(h + 1) * D],
                                attT[:, mc * 512:(mc + 1) * 512],
                                start=(kt == 0), stop=(kt == MT - 1),
                            )
                    # (d) normalize columns by 1/rowsum, store head output^T
                    for mc in range(KC):
                        nc.vector.tensor_tensor(
                            outT[0:64, h, mc * 512:(mc + 1) * 512],
                            ots[mc][:], rbc[0:64, mc * 512:(mc + 1) * 512],
                            mybir.AluOpType.mult,
                        )

            # ---- Phase 3: output projection (partial; host sums cores) ----
            with (
                tc.tile_pool(name="osb", bufs=3) as osb,
                tc.tile_pool(name="po", bufs=4, space="PSUM") as po,
            ):
                for mt in range(MT):
                    oc = osb.tile([128, E], F32, tag="oc")
                    for nh in range(2):
                        p = po.tile([128, 512], F32, tag="po")
                        for h in range(GH):
                            nc.tensor.matmul(
                                p[:],
                                outT[0:64, h, mt * 128:(mt + 1) * 128],
                                wo_sb[:, h, nh * 512:(nh + 1) * 512],
                                start=(h == 0), stop=(h == GH - 1),
                            )
                        nc.vector.tensor_copy(
                            oc[:, nh * 512:(nh + 1) * 512], p[:])
                    nc.sync.dma_start(
                        outp_d[mt * 128:(mt + 1) * 128, :], oc[:])

    nc.compile()
    return nc


def _get_nc():
    if "nc" not in _cache:
        _cache["nc"] = _build()
    return _cache["nc"]


def _make_in_maps(query, key, value, Wq, bq, Wk, bk, Wv, bv, Wo, bo):
    import ml_dtypes
    bf16 = ml_dtypes.bfloat16
    f32 = np.float32
    query = np.asarray(query, f32)
    key = np.asarray(key, f32)
    value = np.asarray(value, f32)
    Wq, Wk, Wv, Wo = (np.asarray(a, f32) for a in (Wq, Wk, Wv, Wo))
    bq, bk = np.asarray(bq, f32), np.asarray(bk, f32)

    xT = {}
    for b in range(B):
        xT[b] = (
            np.ascontiguousarray(query[b].T.astype(bf16)),
            np.ascontiguousarray(key[b].T.astype(bf16)),
            np.ascontiguousarray(value[b].T.astype(bf16)),
        )
    in_maps = []
    for core in range(NCORES):
        b, g = core // (NCORES // B), core % (NCORES // B)
        cols = slice(C * g, C * (g + 1))
        xq, xk, xv = xT[b]
        in_maps.append({
            "xq": xq, "xk": xk, "xv": xv,
            "wq": np.ascontiguousarray(Wq[cols, :].T.astype(bf16)),
            "wk": np.ascontiguousarray(Wk[cols, :].T.astype(bf16)),
            "wv": np.ascontiguousarray(Wv[cols, :].T.astype(bf16)),
            "wo": np.ascontiguousarray(Wo[:, cols].T.reshape(GH, D, E).astype(bf16)),
            "bq2": np.ascontiguousarray(bq[cols].reshape(2, 128).T),
            "bk2": np.ascontiguousarray(bk[cols].reshape(2, 128).T),
        })
    return in_maps


def kernel_run(trace=False, **inputs):
    """Returns ((output, attn_weights), exec_time_ns_or_None)."""
    from concourse.bass_utils import run_bass_kernel_spmd
    nc = _get_nc()
    in_maps = _make_in_maps(**inputs)
    r = run_bass_kernel_spmd(nc, in_maps, list(range(NCORES)), trace=trace)
    res = r.results
    f32 = np.float32
    out = np.zeros((B, S, E), f32)
    attn = np.empty((B, H, S, S), f32)
    for core in range(NCORES):
        b, g = core // (NCORES // B), core % (NCORES // B)
        out[b] += res[core]["outp"]
        attn[b, GH * g:GH * (g + 1)] = res[core]["attn"]
    bv = np.asarray(inputs["bv"], f32)
    bo = np.asarray(inputs["bo"], f32)
    Wo = np.asarray(inputs["Wo"], f32)
    out += (bv @ Wo.T + bo)[None, None, :]
    return (out, attn), r.exec_time_ns


def kernel(**inputs):
    return kernel_run(trace=False, **inputs)[0]
